# revision 4
# baseline (speedup 1.0000x reference)
"""Trainium2 Bass kernel for the EntropyBottleneck likelihood problem.

Reference computation (per channel c, per position n):
    lower = MLP_c(x - 0.5), upper = MLP_c(x + 0.5)
    likelihood = sigmoid(upper) - sigmoid(lower)
where MLP_c is a 5-layer (1->3->3->3->3->1) MLP with softplus-reparametrized
weights and `h + tanh(t)*tanh(h)` gating between layers.

The gate factors t0..t3 are zero in this problem instance, which makes every
gate an exact no-op (tanh(0) * tanh(h) == 0 bitwise).  The MLP is then a chain
of affine maps, so per channel it collapses to a single scalar affine:
    chain_c(x) = a_c * x + beta_c
with a_c / beta_c computed on host in float64 from the (tiny) weight tensors.
The device kernel is then purely memory-bound elementwise work:
    lower = a*x + (beta - 0.5a);  upper = a*x + (beta + 0.5a)
    likelihood = sigmoid(upper) - sigmoid(lower)

Sharding: channels are split across the 8 NeuronCores (24 each) -- pure data
parallelism, no communication.  Per core the (24, 262144) channel slice is
viewed as (384, 16384): row r holds positions of channel r//16.  This makes
the global (8*384, 16384) input exactly x.reshape(3072, 16384) -- a zero-copy
view -- and likewise the gathered outputs reshape straight back to
(192, 1, 262144).  Per-channel scalars arrive as a small (384, 4) coefficient
tensor used as per-partition scalar operands.

If a nonzero gate factor ever shows up, we fall back to a numpy implementation
of the full reference semantics (correct for arbitrary inputs).
"""

import numpy as np

C = 192
N = 262144
NCORES = 8
CPC = C // NCORES  # 24 channels per core
H = 16  # rows per channel on a core
R = CPC * H  # 384 rows per core
TPC = N // H  # 16384 positions per row
P = 128
G = R // P  # 3 partition groups
FREE = 2048  # tile free-dim
NT = TPC // FREE  # 8 tiles per group

_CACHE = {}


def _build_fast_nc():
    import concourse.mybir as mybir
    from concourse import bacc
    from concourse.tile import TileContext

    f32 = mybir.dt.float32
    nc = bacc.Bacc(
        "TRN2",
        target_bir_lowering=False,
        debug=False,
        num_devices=NCORES,
    )
    x = nc.dram_tensor("x", [R, TPC], f32, kind="ExternalInput").ap()
    coef = nc.dram_tensor("coef", [R, 4], f32, kind="ExternalInput").ap()
    lo = nc.dram_tensor("lo", [R, TPC], f32, kind="ExternalOutput").ap()
    up = nc.dram_tensor("up", [R, TPC], f32, kind="ExternalOutput").ap()
    lk = nc.dram_tensor("lk", [R, TPC], f32, kind="ExternalOutput").ap()

    sig = mybir.ActivationFunctionType.Sigmoid
    with TileContext(nc) as tc:
        with (
            tc.tile_pool(name="cpool", bufs=1) as cpool,
            tc.tile_pool(name="xpool", bufs=4) as xpool,
            tc.tile_pool(name="lopool", bufs=3) as lopool,
            tc.tile_pool(name="uppool", bufs=3) as uppool,
            tc.tile_pool(name="slpool", bufs=2) as slpool,
            tc.tile_pool(name="supool", bufs=2) as supool,
            tc.tile_pool(name="lkpool", bufs=3) as lkpool,
        ):
            coefs = []
            for g in range(G):
                ct = cpool.tile([P, 4], f32, tag=f"coef{g}")
                nc.sync.dma_start(out=ct[:], in_=coef[g * P : (g + 1) * P, :])
                coefs.append(ct)
            for g in range(G):
                a = coefs[g][:, 0:1]
                kl = coefs[g][:, 1:2]
                ku = coefs[g][:, 2:3]
                rows = slice(g * P, (g + 1) * P)
                for t in range(NT):
                    cols = slice(t * FREE, (t + 1) * FREE)
                    xt = xpool.tile([P, FREE], f32)
                    nc.sync.dma_start(out=xt[:], in_=x[rows, cols])
                    lot = lopool.tile([P, FREE], f32)
                    nc.vector.tensor_scalar(
                        out=lot[:],
                        in0=xt[:],
                        scalar1=a,
                        scalar2=kl,
                        op0=mybir.AluOpType.mult,
                        op1=mybir.AluOpType.add,
                    )
                    upt = uppool.tile([P, FREE], f32)
                    nc.vector.tensor_scalar(
                        out=upt[:],
                        in0=xt[:],
                        scalar1=a,
                        scalar2=ku,
                        op0=mybir.AluOpType.mult,
                        op1=mybir.AluOpType.add,
                    )
                    slt = slpool.tile([P, FREE], f32)
                    nc.scalar.activation(out=slt[:], in_=xt[:], func=sig, bias=kl, scale=a)
                    sut = supool.tile([P, FREE], f32)
                    nc.scalar.activation(out=sut[:], in_=xt[:], func=sig, bias=ku, scale=a)
                    lkt = lkpool.tile([P, FREE], f32)
                    nc.vector.tensor_sub(out=lkt[:], in0=sut[:], in1=slt[:])
                    nc.sync.dma_start(out=lo[rows, cols], in_=lot[:])
                    nc.sync.dma_start(out=up[rows, cols], in_=upt[:])
                    nc.sync.dma_start(out=lk[rows, cols], in_=lkt[:])
    nc.compile()
    return nc


def _io_names(nc):
    import concourse.mybir as mybir

    in_names, out_names, out_avals = [], [], []
    import jax

    for alloc in nc.m.functions[0].allocations:
        if not isinstance(alloc, mybir.MemoryLocationSet):
            continue
        if not alloc.memorylocations:
            continue
        name = alloc.memorylocations[0].name
        if alloc.kind == "ExternalInput":
            in_names.append(name)
        elif alloc.kind == "ExternalOutput":
            out_names.append(name)
            out_avals.append(
                jax.core.ShapedArray(
                    tuple(alloc.tensor_shape), mybir.dt.np(alloc.dtype)
                )
            )
    return tuple(in_names), tuple(out_names), tuple(out_avals)


def get_runner():
    """Build (once) and return (sharded_fn, mesh, out_names).

    sharded_fn takes the GLOBAL (n_cores*R, ...) arrays for each input and
    returns global output arrays, executing the Bass NEFF on 8 cores.
    """
    if "runner" in _CACHE:
        return _CACHE["runner"]

    import jax
    from jax.sharding import Mesh, PartitionSpec
    from jax.experimental.shard_map import shard_map

    from concourse import bass2jax

    bass2jax.install_neuronx_cc_hook()

    nc = _build_fast_nc()
    in_names, out_names, out_avals = _io_names(nc)
    partition_name = nc.partition_id_tensor.name if nc.partition_id_tensor else None
    user_in_names = tuple(n for n in in_names if n != partition_name)
    assert user_in_names == ("x", "coef"), user_in_names
    # partition_id is supplied last via PartitionIdOp (see run_bass_via_pjrt)
    bind_in_names = user_in_names + ((partition_name,) if partition_name else ())

    def _body(*args):
        operands = list(args)
        if partition_name is not None:
            operands.append(bass2jax.partition_id_tensor())
        outs = bass2jax._bass_exec_p.bind(
            *operands,
            out_avals=out_avals,
            in_names=bind_in_names,
            out_names=out_names,
            lowering_input_output_aliases=(),
            sim_require_finite=True,
            sim_require_nnan=True,
            nc=nc,
        )
        return tuple(outs)

    devices = jax.devices()[:NCORES]
    assert len(devices) == NCORES, f"need {NCORES} devices, got {len(jax.devices())}"
    mesh = Mesh(np.asarray(devices), ("core",))
    spec = PartitionSpec("core")
    sharded = jax.jit(
        shard_map(
            _body,
            mesh=mesh,
            in_specs=(spec,) * len(user_in_names),
            out_specs=(spec,) * len(out_names),
            check_rep=False,
        )
    )
    _CACHE["runner"] = (sharded, mesh, out_names)
    return _CACHE["runner"]


def _softplus64(m):
    return np.logaddexp(0.0, m.astype(np.float64))


def _collapse_affine(ms, bs):
    """Fold the gate-free affine chain into per-channel (a, beta)."""
    A = _softplus64(ms[0])  # (C, 3, 1)
    Bv = bs[0].astype(np.float64)  # (C, 3, 1)
    for i in range(1, 5):
        Mi = _softplus64(ms[i])
        A = Mi @ A
        Bv = Mi @ Bv + bs[i].astype(np.float64)
    return A[:, 0, 0], Bv[:, 0, 0]  # (C,), (C,)


def _numpy_reference(x, ms, bs, ts):
    """Full-semantics fallback (handles nonzero gate factors)."""

    def softplus32(v):
        return np.logaddexp(np.float32(0.0), v).astype(np.float32)

    def chain(h):
        for i in range(5):
            h = np.matmul(softplus32(ms[i]), h) + bs[i]
            if i < 4:
                h = h + np.tanh(ts[i]) * np.tanh(h)
        return h

    half = np.float32(0.5)
    lower = chain(x - half)
    upper = chain(x + half)

    def sigmoid(v):
        return (np.float32(1.0) / (np.float32(1.0) + np.exp(-v))).astype(np.float32)

    likelihood = sigmoid(upper) - sigmoid(lower)
    return likelihood, lower, upper


def make_global_inputs(inputs):
    """Host-side prep: returns (x_glob, coef_glob) global arrays."""
    x = np.ascontiguousarray(np.asarray(inputs["inputs"], dtype=np.float32))
    ms = [np.asarray(inputs[f"m{i}"], dtype=np.float32) for i in range(5)]
    bs = [np.asarray(inputs[f"b{i}"], dtype=np.float32) for i in range(5)]
    a, beta = _collapse_affine(ms, bs)
    coef_c = np.zeros((C, 4), dtype=np.float32)
    coef_c[:, 0] = a.astype(np.float32)
    coef_c[:, 1] = (beta - 0.5 * a).astype(np.float32)
    coef_c[:, 2] = (beta + 0.5 * a).astype(np.float32)
    coef_glob = np.repeat(coef_c, H, axis=0)  # (3072, 4)
    x_glob = x.reshape(NCORES * R, TPC)  # zero-copy view
    return x_glob, coef_glob


def kernel(**inputs):
    x = np.asarray(inputs["inputs"], dtype=np.float32)
    ts = [np.asarray(inputs[f"t{i}"], dtype=np.float32) for i in range(4)]
    assert x.shape == (C, 1, N)

    if any(np.any(t) for t in ts):
        ms = [np.asarray(inputs[f"m{i}"], dtype=np.float32) for i in range(5)]
        bs = [np.asarray(inputs[f"b{i}"], dtype=np.float32) for i in range(5)]
        return _numpy_reference(x, ms, bs, ts)

    x_glob, coef_glob = make_global_inputs(inputs)
    sharded, mesh, out_names = get_runner()
    outs = sharded(x_glob, coef_glob)
    by_name = dict(zip(out_names, outs))
    like = np.asarray(by_name["lk"]).reshape(C, 1, N)
    lo = np.asarray(by_name["lo"]).reshape(C, 1, N)
    up = np.asarray(by_name["up"]).reshape(C, 1, N)
    return like, lo, up


# revision 10
# speedup vs baseline: 288.6401x; 288.6401x over previous
"""Trainium2 Bass kernel for the EntropyBottleneck likelihood problem.

Reference computation (per channel c, per position n):
    lower = MLP_c(x - 0.5), upper = MLP_c(x + 0.5)
    likelihood = sigmoid(upper) - sigmoid(lower)
where MLP_c is a 5-layer (1->3->3->3->3->1) MLP with softplus-reparametrized
weights and `h + tanh(t)*tanh(h)` gating between layers.

The gate factors t0..t3 are zero in this problem instance, which makes every
gate an exact no-op (tanh(0) * tanh(h) == 0 bitwise).  The MLP is then a chain
of affine maps, so per channel it collapses to a single scalar affine:
    chain_c(x) = a_c * x + beta_c
with a_c / beta_c computed on host in float64 from the (tiny) weight tensors.
The device kernel is then purely memory-bound elementwise work:
    lower = a*x + (beta - 0.5a);  upper = a*x + (beta + 0.5a)
    likelihood = sigmoid(upper) - sigmoid(lower)

Sharding: channels are split across the 8 NeuronCores (24 each) -- pure data
parallelism, no communication.  Per core the (24, 262144) channel slice is
viewed as (384, 16384): row r holds positions of channel r//16.  This makes
the global (8*384, 16384) input exactly x.reshape(3072, 16384) -- a zero-copy
view -- and likewise the gathered outputs reshape straight back to
(192, 1, 262144).  Per-channel scalars arrive as a small (384, 4) coefficient
tensor used as per-partition scalar operands.

If a nonzero gate factor ever shows up, we fall back to a numpy implementation
of the full reference semantics (correct for arbitrary inputs).
"""

import numpy as np

C = 192
N = 262144
NCORES = 8
CPC = C // NCORES  # 24 channels per core
H = 16  # rows per channel on a core
R = CPC * H  # 384 rows per core
TPC = N // H  # 16384 positions per row
P = 128
G = R // P  # 3 partition groups
FREE = 2048  # tile free-dim
NT = TPC // FREE  # 8 tiles per group

_CACHE = {}


def _build_fast_nc(reps=1):
    import contextlib

    import concourse.mybir as mybir
    from concourse import bacc
    from concourse.tile import TileContext

    f32 = mybir.dt.float32
    nc = bacc.Bacc(
        "TRN2",
        target_bir_lowering=False,
        debug=False,
        num_devices=NCORES,
    )
    x = nc.dram_tensor("x", [R, TPC], f32, kind="ExternalInput").ap()
    coef = nc.dram_tensor("coef", [R, 4], f32, kind="ExternalInput").ap()
    lo = nc.dram_tensor("lo", [R, TPC], f32, kind="ExternalOutput").ap()
    up = nc.dram_tensor("up", [R, TPC], f32, kind="ExternalOutput").ap()
    lk = nc.dram_tensor("lk", [R, TPC], f32, kind="ExternalOutput").ap()

    with TileContext(nc) as tc:
        with tc.tile_pool(name="cpool", bufs=1) as cpool:
            coefs = []
            for g in range(G):
                ct = cpool.tile([P, 4], f32, tag=f"coef{g}")
                nc.sync.dma_start(out=ct[:], in_=coef[g * P : (g + 1) * P, :])
                coefs.append(ct)
            rep_loop = tc.For_i(0, reps, 1) if reps > 1 else contextlib.nullcontext()
            with rep_loop:
                _emit_body(nc, tc, mybir, coefs, x, lo, up, lk)
    nc.compile()
    return nc


def _emit_body(nc, tc, mybir, coefs, x, lo, up, lk):
    f32 = mybir.dt.float32
    sig = mybir.ActivationFunctionType.Sigmoid
    with (
        tc.tile_pool(name="xpool", bufs=4) as xpool,
        tc.tile_pool(name="lopool", bufs=3) as lopool,
        tc.tile_pool(name="uppool", bufs=3) as uppool,
        tc.tile_pool(name="slpool", bufs=2) as slpool,
        tc.tile_pool(name="supool", bufs=2) as supool,
        tc.tile_pool(name="lkpool", bufs=3) as lkpool,
    ):
        for g in range(G):
            a = coefs[g][:, 0:1]
            kl = coefs[g][:, 1:2]
            ku = coefs[g][:, 2:3]
            rows = slice(g * P, (g + 1) * P)
            for t in range(NT):
                cols = slice(t * FREE, (t + 1) * FREE)
                xt = xpool.tile([P, FREE], f32)
                nc.sync.dma_start(out=xt[:], in_=x[rows, cols])
                lot = lopool.tile([P, FREE], f32)
                nc.vector.tensor_scalar(
                    out=lot[:],
                    in0=xt[:],
                    scalar1=a,
                    scalar2=kl,
                    op0=mybir.AluOpType.mult,
                    op1=mybir.AluOpType.add,
                )
                upt = uppool.tile([P, FREE], f32)
                nc.vector.tensor_scalar(
                    out=upt[:],
                    in0=xt[:],
                    scalar1=a,
                    scalar2=ku,
                    op0=mybir.AluOpType.mult,
                    op1=mybir.AluOpType.add,
                )
                slt = slpool.tile([P, FREE], f32)
                nc.scalar.activation(out=slt[:], in_=xt[:], func=sig, bias=kl, scale=a)
                sut = supool.tile([P, FREE], f32)
                nc.scalar.activation(out=sut[:], in_=xt[:], func=sig, bias=ku, scale=a)
                lkt = lkpool.tile([P, FREE], f32)
                nc.vector.tensor_sub(out=lkt[:], in0=sut[:], in1=slt[:])
                nc.sync.dma_start(out=lo[rows, cols], in_=lot[:])
                nc.sync.dma_start(out=up[rows, cols], in_=upt[:])
                nc.sync.dma_start(out=lk[rows, cols], in_=lkt[:])


def _io_names(nc):
    import concourse.mybir as mybir

    in_names, out_names, out_avals = [], [], []
    import jax

    for alloc in nc.m.functions[0].allocations:
        if not isinstance(alloc, mybir.MemoryLocationSet):
            continue
        if not alloc.memorylocations:
            continue
        name = alloc.memorylocations[0].name
        if alloc.kind == "ExternalInput":
            in_names.append(name)
        elif alloc.kind == "ExternalOutput":
            out_names.append(name)
            out_avals.append(
                jax.core.ShapedArray(
                    tuple(alloc.tensor_shape), mybir.dt.np(alloc.dtype)
                )
            )
    return tuple(in_names), tuple(out_names), tuple(out_avals)


def get_runner(reps=1):
    """Build (once) and return (sharded_fn, mesh, out_names).

    sharded_fn takes the GLOBAL (n_cores*R, ...) arrays for each input and
    returns global output arrays, executing the Bass NEFF on 8 cores.
    """
    key = ("runner", reps)
    if key in _CACHE:
        return _CACHE[key]

    import jax
    from jax.sharding import Mesh, PartitionSpec
    from jax.experimental.shard_map import shard_map

    from concourse import bass2jax

    bass2jax.install_neuronx_cc_hook()

    nc = _build_fast_nc(reps=reps)
    in_names, out_names, out_avals = _io_names(nc)
    partition_name = nc.partition_id_tensor.name if nc.partition_id_tensor else None
    user_in_names = tuple(n for n in in_names if n != partition_name)
    assert user_in_names == ("x", "coef"), user_in_names
    # partition_id is supplied last via PartitionIdOp (see run_bass_via_pjrt)
    bind_in_names = user_in_names + ((partition_name,) if partition_name else ())

    def _body(*args):
        operands = list(args)
        if partition_name is not None:
            operands.append(bass2jax.partition_id_tensor())
        outs = bass2jax._bass_exec_p.bind(
            *operands,
            out_avals=out_avals,
            in_names=bind_in_names,
            out_names=out_names,
            lowering_input_output_aliases=(),
            sim_require_finite=True,
            sim_require_nnan=True,
            nc=nc,
        )
        return tuple(outs)

    devices = jax.devices()[:NCORES]
    assert len(devices) == NCORES, f"need {NCORES} devices, got {len(jax.devices())}"
    mesh = Mesh(np.asarray(devices), ("core",))
    spec = PartitionSpec("core")
    sharded = jax.jit(
        shard_map(
            _body,
            mesh=mesh,
            in_specs=(spec,) * len(user_in_names),
            out_specs=(spec,) * len(out_names),
            check_rep=False,
        )
    )
    _CACHE[key] = (sharded, mesh, out_names)
    return _CACHE[key]


def _softplus64(m):
    return np.logaddexp(0.0, m.astype(np.float64))


def _collapse_affine(ms, bs):
    """Fold the gate-free affine chain into per-channel (a, beta)."""
    A = _softplus64(ms[0])  # (C, 3, 1)
    Bv = bs[0].astype(np.float64)  # (C, 3, 1)
    for i in range(1, 5):
        Mi = _softplus64(ms[i])
        A = Mi @ A
        Bv = Mi @ Bv + bs[i].astype(np.float64)
    return A[:, 0, 0], Bv[:, 0, 0]  # (C,), (C,)


def _numpy_reference(x, ms, bs, ts):
    """Full-semantics fallback (handles nonzero gate factors)."""

    def softplus32(v):
        return np.logaddexp(np.float32(0.0), v).astype(np.float32)

    def chain(h):
        for i in range(5):
            h = np.matmul(softplus32(ms[i]), h) + bs[i]
            if i < 4:
                h = h + np.tanh(ts[i]) * np.tanh(h)
        return h

    half = np.float32(0.5)
    lower = chain(x - half)
    upper = chain(x + half)

    def sigmoid(v):
        return (np.float32(1.0) / (np.float32(1.0) + np.exp(-v))).astype(np.float32)

    likelihood = sigmoid(upper) - sigmoid(lower)
    return likelihood, lower, upper


def make_global_inputs(inputs):
    """Host-side prep: returns (x_glob, coef_glob) global arrays."""
    x = np.ascontiguousarray(np.asarray(inputs["inputs"], dtype=np.float32))
    ms = [np.asarray(inputs[f"m{i}"], dtype=np.float32) for i in range(5)]
    bs = [np.asarray(inputs[f"b{i}"], dtype=np.float32) for i in range(5)]
    a, beta = _collapse_affine(ms, bs)
    coef_c = np.zeros((C, 4), dtype=np.float32)
    coef_c[:, 0] = a.astype(np.float32)
    coef_c[:, 1] = (beta - 0.5 * a).astype(np.float32)
    coef_c[:, 2] = (beta + 0.5 * a).astype(np.float32)
    coef_glob = np.repeat(coef_c, H, axis=0)  # (3072, 4)
    x_glob = x.reshape(NCORES * R, TPC)  # zero-copy view
    return x_glob, coef_glob


def kernel(**inputs):
    x = np.asarray(inputs["inputs"], dtype=np.float32)
    ts = [np.asarray(inputs[f"t{i}"], dtype=np.float32) for i in range(4)]
    assert x.shape == (C, 1, N)

    if any(np.any(t) for t in ts):
        ms = [np.asarray(inputs[f"m{i}"], dtype=np.float32) for i in range(5)]
        bs = [np.asarray(inputs[f"b{i}"], dtype=np.float32) for i in range(5)]
        return _numpy_reference(x, ms, bs, ts)

    x_glob, coef_glob = make_global_inputs(inputs)
    sharded, mesh, out_names = get_runner()
    outs = sharded(x_glob, coef_glob)
    by_name = dict(zip(out_names, outs))
    like = np.asarray(by_name["lk"]).reshape(C, 1, N)
    lo = np.asarray(by_name["lo"]).reshape(C, 1, N)
    up = np.asarray(by_name["up"]).reshape(C, 1, N)
    return like, lo, up
